# revision 30
# baseline (speedup 1.0000x reference)
"""Trainium2 Bass kernel for a CGNS block (GNN message passing).

Math: the reference builds A = a a^T + I (rank-1 + identity), L = D^-1/2 A D^-1/2,
then out = relu(BN(conv1x1(cat[x@A, (L@x^T)^T]))).  Exploiting the rank-1
structure, with a = relu(tanh(w)), S = sum(a), d_n = 1/sqrt(a_n*S + 1),
u = d*a, s0 = x@a, s1 = x@u, the whole block collapses to

  y[:, n] = W1~ x[:, n] + d2[n] * (W2~ x[:, n]) + a[n] v1 + u[n] v2 + b~
  out     = relu(y)

where W~ are the BN-folded conv weights, v1 = W1~ s0, v2 = W2~ s1.  No [N,N]
matrix is ever materialized.

Sharding: 8 cores; core i handles batch b = i//2, half h = i%2 of the N=4096
node dim (2048 columns each).  Each core reads the full x[b] once in
transposed layout (for the s0/s1 reduction, which needs all of N) and its own
half in natural layout (for the main matmuls).  n-chunks are rolled per-core
so that chunks 0..15 are always the core's own half -> identical SPMD program.

On-device layout is output-transposed (n on partitions) so d2/a/u are
per-partition scalars.

v2 scheduling (from trace analysis of the 32us baseline):
 - single tanh + single sqrt on the scalar engine (2 LUT loads, not 4); the
   a/u/ones rows of xa come from a PE transpose of the column-layout values
   instead of a recomputed row path gated on a late wrow DMA.
 - v1/v2 land in wAB via a matmul aimed at PSUM partitions 64:66 plus a
   partition-aligned vector copy (no SBUF->SBUF DMA roundtrip).
 - DMA: wcol first on the sync HW queue, wvb early on the scalar HW queue
   (issued between tanh and the sqrt table load), xt split sync/scalar/SWDGE,
   xh halves last (only gate the late main-matmul weight loads), outputs
   alternate across both HW queues.
 - epilogue scalar_tensor_tensor reads both tensor operands straight from
   PSUM and is split across the vector and gpsimd engines.
"""

import numpy as np

import concourse.bacc as bacc
import concourse.bass as bass
import concourse.tile as tile
from concourse import masks, mybir

FP = mybir.dt.float32
FPR = mybir.dt.float32r
B, C, N = 4, 64, 4096
NH = N // 2          # columns per core
JH = NH // 128       # 16 chunks per core half
JF = N // 128        # 32 chunks full N
BN_EPS = 1e-5


def build_nc():
    # Bacc (not raw Bass): its compile() pipeline legalizes TRN2's
    # one-wait-per-instruction constraint (move_matmul_waits_to_ldweights,
    # generate_event_semaphores) which Tile-emitted multi-waits require.
    nc = bacc.Bacc()
    AF = mybir.ActivationFunctionType
    OP = mybir.AluOpType
    AX = mybir.AxisListType

    # DRAM I/O (per-core shards supplied via in_maps)
    xt = nc.dram_tensor("xt", [128, JF, C], FPR, kind="ExternalInput")
    xh = nc.dram_tensor("xh", [C, NH], FPR, kind="ExternalInput")
    wcol = nc.dram_tensor("wcol", [128, 32], FP, kind="ExternalInput")
    wvb = nc.dram_tensor("wvb", [67, 2 * C], FPR, kind="ExternalInput")
    out = nc.dram_tensor("out", [128, JH, C], FP, kind="ExternalOutput")

    with tile.TileContext(nc) as tc:
        with (
            tc.tile_pool(name="sb", bufs=1) as sb,
            tc.tile_pool(name="ps", bufs=1, space="PSUM") as ps,
        ):
            # SBUF tiles
            xt_sb = sb.tile([128, JF, C], FPR, name="xt_sb")
            # xa: rows 0:64 = x half (natural layout), 64 = a, 65 = u, 66 = 1
            xa = sb.tile([67, NH], FPR, name="xa")
            wcol_sb = sb.tile([128, 32], FP, name="wcol_sb")
            # wAB rows 0:64 = [W1~T | W2~T] (host), 64 = v1, 65 = v2 (device;
            # host preloads zeros so the unused column halves stay zero),
            # 66 = [b~ | 0] (host)
            wAB = sb.tile([67, 2 * C], FPR, name="wAB")
            tcol = sb.tile([128, 32], FP, name="tcol")
            dcol = sb.tile([128, 32], FP, name="dcol")
            d2col = sb.tile([128, 32], FP, name="d2col")
            # aucol: cols 0:32 = a (col layout), 32:64 = u
            aucol = sb.tile([128, 2 * 32], FPR, name="aucol")
            # auo48: transpose input [a_own | u_own | ones]
            auo48 = sb.tile([128, 48], FP, name="auo48")
            ident = sb.tile([128, 128], FP, name="ident")
            ones_fp = sb.tile([128, 128], FP, name="ones_fp")
            ones = sb.tile([128, 128], FPR, name="ones")
            apart = sb.tile([128, 1], FP, name="apart")
            apart_r = sb.tile([128, 2], FPR, name="apart_r")
            rows48 = sb.tile([128, 128], FP, name="rows48")
            sS = sb.tile([128, 1], FP, name="sS")
            # v-matmul stationaries: 66 cols of zeros except col 64 = s0 (A)
            # resp. col 65 = s1 (B), so v1/v2 land at PSUM partitions 64/65
            # without PE tile_position games.
            s0A = sb.tile([C, 66], FPR, name="s0A")
            s1B = sb.tile([C, 66], FPR, name="s1B")
            y1 = sb.tile([128, JH * C], FP, name="y1")
            yo = sb.tile([128, JH * C], FP, name="yo")

            # PSUM tiles (each padded to a bank; 8 total = 8 banks)
            p_sm = ps.tile([128, 2], FP, name="p_sm")
            p_s = ps.tile([C, 2], FP, name="p_s")
            p_v = ps.tile([66, 2 * C], FP, name="p_v")
            p_rows = ps.tile([48, 128], FP, name="p_rows")
            p_yq = [ps.tile([128, 512], FP, name=f"p_yq_{g}") for g in range(4)]

            # ---- DMA issues -------------------------------------------------
            # sync HW queue: wcol (gates the whole scalar chain) first, then
            # the own-half of xt, then the first xh half (only gates the late
            # main-matmul weight loads).
            nc.sync.dma_start(wcol_sb[:], wcol[:])
            nc.sync.dma_start(xt_sb[:, 0:16, :], xt[:, 0:16, :])
            nc.sync.dma_start(xa[0:C, 0:1024], xh[:, 0:1024])
            # SWDGE (gpsimd): identity build first (no data deps), then the
            # last quarter of xt.
            masks.make_identity(nc, ident[:])
            nc.gpsimd.dma_start(xt_sb[:, 24:32, :], xt[:, 24:32, :])

            # ---- scalar engine: tanh, (wvb issue), sqrt --------------------
            # Emission order on the scalar engine controls LUT loads: exactly
            # one tanh table load and one sqrt table load.  The wvb DMA issue
            # sits between them so its transfer overlaps the sqrt table load.
            nc.scalar.activation(tcol[:], wcol_sb[:], AF.Tanh)
            nc.scalar.dma_start(wAB[:], wvb[:])
            # vector chain: a, S partials
            nc.vector.memset(ones_fp[:], 1.0)
            nc.vector.tensor_copy(ones[:], ones_fp[:])
            nc.vector.memset(auo48[:, 32:48], 1.0)
            nc.vector.tensor_scalar_max(aucol[:, 0:32], tcol[:], 0.0)
            nc.vector.tensor_reduce(apart[:], aucol[:, 0:32], axis=AX.X, op=OP.add)
            nc.vector.tensor_copy(apart_r[:, 0:1], apart[:])
            nc.vector.tensor_copy(apart_r[:, 1:2], apart[:])
            # zero-fill the v-matmul stationaries (fp32r-rounded producer)
            nc.vector.tensor_scalar_mul(s0A[:], ident[0:C, 0:66], 0.0)
            nc.vector.tensor_scalar_mul(s1B[:], ident[0:C, 0:66], 0.0)
            # S broadcast to all partitions via ones matmul (fp32r single pass)
            nc.tensor.matmul(p_sm[:], ones[:], apart_r[:], start=True, stop=True)
            nc.vector.tensor_copy(sS[:], p_sm[:, 0:1])
            # t = a*S + 1 ; d2 = 1/t ; d = sqrt(d2) ; u = d*a
            nc.vector.tensor_scalar(
                tcol[:], aucol[:, 0:32], sS[:], 1.0, op0=OP.mult, op1=OP.add
            )
            nc.vector.reciprocal(d2col[:], tcol[:])
            nc.scalar.sqrt(dcol[:], d2col[:])
            # remaining scalar-queue DMA issues go after the sqrt so they don't
            # delay the LUT load: xt quarter 16:24, then the second xh half.
            nc.scalar.dma_start(xt_sb[:, 16:24, :], xt[:, 16:24, :])
            nc.scalar.dma_start(xa[0:C, 1024:2048], xh[:, 1024:2048])
            nc.vector.tensor_mul(aucol[:, 32:64], dcol[:], aucol[:, 0:32])
            # transpose input: [a_own | u_own | ones]
            nc.vector.tensor_copy(auo48[:, 0:16], aucol[:, 0:16])
            nc.vector.tensor_copy(auo48[:, 16:32], aucol[:, 32:48])

            # ---- a/u/ones rows of xa via PE transpose ----------------------
            # p_rows[j, p] = auo48[p, j]: rows 0:16 = a chunks, 16:32 = u,
            # 32:48 = ones.  One DMA scatters them into xa rows 64:67.
            nc.tensor.matmul(
                p_rows[:], auo48[:], ident[:], is_transpose=True,
                start=True, stop=True,
            )
            nc.scalar.copy(rows48[0:48, :], p_rows[:])
            nc.gpsimd.dma_start(
                xa[64:67, :].rearrange("r (j p) -> r j p", p=128),
                rows48[0:48, :],
            )

            # ---- s0/s1 reduction over full N (PE, accumulate in PSUM) ------
            au_v = aucol[:].rearrange("p (s c) -> p s c", s=2)
            for j in range(JF):
                nc.tensor.matmul(
                    p_s[:],
                    xt_sb[:, j, :],
                    au_v[:, :, j],
                    start=(j == 0),
                    stop=(j == JF - 1),
                )
            nc.vector.tensor_copy(s0A[:, 64:65], p_s[:, 0:1])
            nc.vector.tensor_copy(s1B[:, 65:66], p_s[:, 1:2])

            # ---- v1/v2: rows 64/65 of the [66, .] outputs are s0*W1 / s1*W2
            # (all other rows zero), then one partition-aligned copy into wAB
            # rows 64:65: [v1 | 0] and [0 | v2].
            nc.tensor.matmul(
                p_v[0:66, 0:C], s0A[:], wAB[0:C, 0:C], start=True, stop=True
            )
            nc.tensor.matmul(
                p_v[0:66, C : 2 * C], s1B[:], wAB[0:C, C : 2 * C],
                start=True, stop=True,
            )
            nc.vector.tensor_copy(wAB[64:66, :], p_v[64:66, :])

            # ---- main matmuls: one [67,128]x[67,128] mm per chunk.
            # out columns 0:64 = y1 (conv1 + rank-1 + bias), 64:128 = q (conv2)
            for j in range(JH):
                grp, jj = divmod(j, 4)
                nc.tensor.matmul(
                    p_yq[grp][:, 128 * jj : 128 * (jj + 1)],
                    xa[:, 128 * j : 128 * (j + 1)],
                    wAB[:],
                    start=True, stop=True,
                )

            # ---- epilogue: yo = relu(q * d2 + y1).  The STT may read only one
            # PSUM operand, so the y1 half is first evacuated per group on the
            # (otherwise idle) scalar engine; STTs split across vector/gpsimd.
            for g in range(4):
                nc.scalar.copy(
                    y1[:, 256 * g : 256 * (g + 1)].rearrange(
                        "p (j c) -> p j c", c=C
                    ),
                    p_yq[g][:].rearrange("p (j c) -> p j c", c=2 * C)[:, :, 0:C],
                )
            for j in range(JH):
                g, jj = divmod(j, 4)
                nc.vector.scalar_tensor_tensor(
                    yo[:, C * j : C * (j + 1)],
                    p_yq[g][:, 128 * jj + C : 128 * jj + 2 * C],
                    d2col[:, j : j + 1],
                    y1[:, C * j : C * (j + 1)],
                    op0=OP.mult,
                    op1=OP.add,
                )
            for g in range(4):
                nc.scalar.activation(
                    yo[:, 256 * g : 256 * (g + 1)], yo[:, 256 * g : 256 * (g + 1)],
                    AF.Relu,
                )
                eng = nc.scalar if g % 2 == 0 else nc.sync
                eng.dma_start(
                    out[:, 4 * g : 4 * (g + 1), :],
                    yo[:, 256 * g : 256 * (g + 1)].rearrange("p (j c) -> p j c", c=C),
                )
    nc.compile()
    return nc


def make_in_maps(x, w, conv_w, conv_b, bn_gamma, bn_beta, bn_mean, bn_var):
    x = np.asarray(x, np.float32)
    w = np.asarray(w, np.float32)
    conv_w = np.asarray(conv_w, np.float32)
    conv_b = np.asarray(conv_b, np.float32)
    bn_gamma = np.asarray(bn_gamma, np.float32)
    bn_beta = np.asarray(bn_beta, np.float32)
    bn_mean = np.asarray(bn_mean, np.float32)
    bn_var = np.asarray(bn_var, np.float32)

    scale = bn_gamma / np.sqrt(bn_var + BN_EPS)
    wmat = conv_w * scale[:, None]                       # [64, 128] BN-folded
    w1t = np.ascontiguousarray(wmat[:, :C].T)            # [c, o]
    w2t = np.ascontiguousarray(wmat[:, C:].T)
    wvb = np.zeros((67, 2 * C), np.float32)
    wvb[0:C] = np.concatenate([w1t, w2t], axis=1)
    wvb[66, :C] = conv_b * scale + bn_beta - bn_mean * scale

    in_maps = []
    for i in range(8):
        b, h = divmod(i, 2)
        xb = x[b, :, :, 0]                               # [64, 4096]
        order = np.roll(np.arange(JF), -JH * h)          # own half first
        xt_jpc = np.ascontiguousarray(xb.T).reshape(JF, 128, C)
        xt_pjc = np.ascontiguousarray(xt_jpc[order].transpose(1, 0, 2))
        xhb = np.ascontiguousarray(xb[:, NH * h : NH * (h + 1)])
        wcol = np.ascontiguousarray(w[b].reshape(JF, 128).T[:, order])
        in_maps.append(
            {
                "xt": xt_pjc,
                "xh": xhb,
                "wcol": wcol,
                "wvb": wvb,
            }
        )
    return in_maps


def assemble_out(results):
    out = np.empty((B, C, N), np.float32)
    for i in range(8):
        b, h = divmod(i, 2)
        blk = np.asarray(results[i]["out"])              # [128, 16, 64]
        y_half = blk.transpose(1, 0, 2).reshape(NH, C)   # row = 128*j + p
        out[b, :, NH * h : NH * (h + 1)] = y_half.T
    return out[..., None]


_NC = None


def kernel(**inputs):
    global _NC
    from concourse.bass_utils import run_bass_kernel_spmd

    if _NC is None:
        _NC = build_nc()
    in_maps = make_in_maps(**inputs)
    res = run_bass_kernel_spmd(_NC, in_maps, list(range(8)))
    return assemble_out(res.results)
